# revision 37
# baseline (speedup 1.0000x reference)
"""Supervised contrastive loss (nn_Batch_CL) on 8 Trainium2 NeuronCores.

Math (per the reference):
  x = l2_normalize(feature_embeds)            # [N, D]
  logits = (x @ x.T) / tau                    # tau = 0.1
  Z_i    = sum_{j != i} exp(logits[i, j])
  S_i    = sum_{j != i, l_j == l_i} logits[i, j]
  per_row_i = S_i / P_i - log Z_i   (P_i = #positives, if > 0)
  loss = -sum(per_row) / n_valid

Strategy (v2): exploit the SYMMETRY of the logits matrix — only the upper
triangle of the 64x64 grid of [128,128] tiles is exp'd (half the N^2 ACT
work).  Each tile contributes its row-sums (ACT accum_out, free) to the Z of
its own rows AND its column-sums (per-tile e-as-stationary PE matmul with a
ones vector, out [128,1] per tile) to the Z of the mirrored rows.

Distribution: circulant chunk assignment.  Global chunk m (of 64) owns tiles
(m, m+d mod 64) for d=0..32 if m<32 else d=0..31 (each unordered chunk pair
covered exactly once).  Core c takes global chunks {c, c+8, ..., c+56}; its
input rows are rotated by 128*c so every core runs the IDENTICAL program on
local chunks {0,8,...,56}.  The wrap (mod 8192 columns) is removed by feeding
xT extended with a 3072-column copy of the first columns.

The exp work is split between ACT (exact spline exp, with free row-sums via
accum_out) and the DVE (Schraudolph bit-trick exp producing bf16 bit patterns
in an int16 tile, rel err ~±3% zero-mean — vanishes in the 8192-term Z sums).

Host does all O(N) work: l2-normalize + transpose + bf16 cast, the
positive-pair sums S = (x @ Msum^T)[i, lab_i] via the class-sum matrix, and
the final assembly (Z = row parts + unrotated col parts - exact diag, log,
positive counts, masked mean).  The device is pure N^2 compute: logits
matmuls, exp + row-sums, colsum matmuls.
"""

import numpy as np
import ml_dtypes

N = 8192
D = 128
N_CORES = 8
NCH = 64                         # global/local 128-row chunks
OWN = [0, 8, 16, 24, 32, 40, 48, 56]   # local chunk ids owned by every core
NOWN = len(OWN)
XT_COLS = N + 3072               # extended (wrap-free) xT width; max span end
PIECE = 1536                     # psum piece width (3 banks)
NCLS = 33
INV_TAU = 10.0

def _width(t):                   # tiles in chunk t's span, incl. diagonal tile
    return 33 if t < 32 else 32

# per-chunk static piece tables: list of (psum_width, col0)
def _pieces(t):
    ws = _width(t) * 128
    out = []
    off = 0
    while off < ws:
        w = min(PIECE, ws - off)
        out.append((w, t * 128 + off))
        off += w
    return out

N_ACT_SLOTS = sum(len(_pieces(t)) for t in OWN)

# Schraudolph fast-exp on DVE: e^(10*s) ~= bitcast_bf16(int16(s*SCH_A + SCH_B)).
# bf16 bits = 128*(127 + log2 y); SCH_A = 128*10*log2(e); the -7.3344 centers
# the piecewise-linear log2 error to zero mean over a uniform mantissa.
SCH_A = 128.0 * INV_TAU * 1.4426950408889634
SCH_B = 128.0 * 127 - 7.3344
DVE_PIECE = 1         # piece index routed through the DVE fast-exp
DVE_CHUNKS = {1, 2, 3, 4, 5, 6}   # chunk indices whose piece 1 goes to DVE
                                  # (0 and 7 stay ACT: chunk 7's tail colsums
                                  # must not wait on the DVE pipeline)

def _touchers(n):
    """Own chunks whose span covers extended column-chunk n (no wrap)."""
    return [t for t in OWN if 1 <= n - t <= _width(t) - 1]

_NC = None

# ---------------------------------------------------------------------------
# Inlined workarounds (kernel.py must be self-contained).
# The local walrus build accepts at most ONE sync-wait per instruction; Tile
# attaches several.  Patch the drain barrier + hoist extra waits onto nops.
# ---------------------------------------------------------------------------

_nop_counter = [0]


def _split_drain_and_barrier(self, tick_clock, wait_clock):
    import bass_rust

    vec = tick_clock.global_clock  # VectorClock
    for proc in range(len(vec)):
        tickv = vec[proc]
        if tickv > 0:
            nop_inst = self.nc.sync.nop(nofuse=True)
            c = bass_rust.ScopedClock()
            c.require_at_least(None, proc, tickv)
            wait_clock.add_sem_waits(nop_inst.ins, c)
    self.nc.sync.drain()
    self.nc.all_engine_barrier()
    assert self.sems is not None
    popped = self.nc._tile_sem_poison_stack.pop()
    assert popped is self._sem_poison
    self.nc.clear_and_free_semaphores(list(self.sems.allocated().values()))
    self.nc.all_engine_barrier()


def _install_tile_patch():
    from concourse import tile as _tile

    _tile.TileContext._drain_and_barrier = _split_drain_and_barrier


def _split_multiwait(nc):
    """Hoist all-but-one sync wait from every instruction onto nops."""
    import concourse.mybir as mybir

    n_hoisted = 0
    for bb in nc.main_func.blocks:
        insns = bb.instructions
        out = []
        changed = False
        for ins in insns:
            si = ins.sync_info
            if si is not None and len(si.on_wait) > 1:
                waits = list(si.on_wait)
                for w in waits[:-1]:
                    _nop_counter[0] += 1
                    nop = mybir.InstEventSemaphore(
                        name=f"hoistnop-{_nop_counter[0]}",
                        engine=ins.engine,
                        sync_info=mybir.SyncInfo(on_wait=[w], on_update=[]),
                    )
                    out.append(nop)
                    n_hoisted += 1
                ins.sync_info = mybir.SyncInfo(
                    on_wait=[waits[-1]], on_update=list(si.on_update)
                )
                changed = True
            out.append(ins)
        if changed:
            bb.instructions = out
    return n_hoisted


def _install_ntff_hook():
    """Synthesize the antenv.axon_hooks module missing from this image so
    run_bass_kernel_spmd(trace=True) can NTFF-profile under axon."""
    import sys
    import types

    if "antenv.axon_hooks" in sys.modules:
        return True
    try:
        import antenv
        from trn_agent_boot.trn_boot import _ntff_profile_via_ctypes
    except ImportError:
        return False
    hook_box = [None]
    mod = types.ModuleType("antenv.axon_hooks")
    mod.set_axon_ntff_profile_hook = lambda h: hook_box.__setitem__(0, h)
    mod.get_axon_ntff_profile_hook = lambda: hook_box[0]
    sys.modules["antenv.axon_hooks"] = mod
    antenv.axon_hooks = mod
    hook = _ntff_profile_via_ctypes("/opt/axon/libaxon_pjrt.so")
    mod.set_axon_ntff_profile_hook(hook)
    return hook is not None


def _build_nc(split_waits=True):
    import concourse.bass as bass
    import concourse.mybir as mybir
    from concourse import tile
    from contextlib import ExitStack

    _install_tile_patch()

    f32 = mybir.dt.float32
    bf16 = mybir.dt.bfloat16

    nc = bass.Bass()
    xT_dram = nc.dram_tensor("xT", [128, XT_COLS], bf16, kind="ExternalInput")
    zact_dram = nc.dram_tensor("zact", [128, N_ACT_SLOTS], f32, kind="ExternalOutput")
    zdve_dram = nc.dram_tensor("zdve", [128, NOWN], f32, kind="ExternalOutput")
    colacc_dram = nc.dram_tensor("colacc", [128, 96], f32, kind="ExternalOutput")

    with tile.TileContext(nc) as tc, ExitStack() as ctx:
        persist = ctx.enter_context(tc.tile_pool(name="persist", bufs=1))

        xT = persist.tile([128, XT_COLS], bf16)
        ones_bf = persist.tile([128, 1], bf16)
        colacc_sb = persist.tile([128, 96], f32)
        Zact = persist.tile([128, N_ACT_SLOTS], f32)
        Zdve = persist.tile([128, NOWN], f32)

        # ---------------- prologue ----------------
        # finer input DMA pieces alternating over the two hardware-DGE queues
        # (SP + ACT); gpsimd's software DGE would contend the DVE SBUF port
        bounds = [0, 512, 1536, 3072, 4608, 6144, 8192, 9728, XT_COLS]
        dma_engines = [nc.sync, nc.scalar]
        for k in range(len(bounds) - 1):
            dma_engines[k % 2].dma_start(
                xT[:, bounds[k]:bounds[k + 1]],
                xT_dram[:, bounds[k]:bounds[k + 1]])
        nc.vector.memset(ones_bf[:], 1.0)

        # ---------------- main loop ----------------
        with (
            tc.tile_pool(name="main_ps", bufs=2, space="PSUM") as main_ps,
            tc.tile_pool(name="strip_ps", bufs=2, space="PSUM") as strip_ps,
            tc.tile_pool(name="ebuf", bufs=2) as ebuf_pool,
            tc.tile_pool(name="i16", bufs=2) as i16_pool,
        ):
            # warm the PE (HAM clock gate) with bare weight loads (junk data,
            # no PSUM use) while the input DMA streams in
            garb = ebuf_pool.tile([128, 128], bf16, tag="warm")
            nc.vector.memset(garb[:], 1.0)
            for _ in range(56):
                nc.tensor.ldweights(weights=garb[:])
            nc.vector.memset(colacc_sb[:], 0.0)

            pending = []  # deferred PE work, finest grain:
                          # ("M", strip, src_ap, col)  one colsum matmul
                          # ("FIN", t, strip, w)       DVE add of a full strip

            def pump(nitems):
                for _ in range(nitems):
                    if not pending:
                        return
                    item = pending.pop(0)
                    if item[0] == "M":
                        _, strip, src_ap, sc = item
                        nc.tensor.matmul(
                            strip[:, sc:sc + 1], src_ap, ones_bf[:],
                            start=True, stop=True,
                        )
                    else:
                        _, t, strip, w = item
                        # colacc[:, t+1 : t+w] += strip
                        nc.vector.tensor_tensor(
                            out=colacc_sb[:, t + 1:t + w],
                            in0=colacc_sb[:, t + 1:t + w],
                            in1=strip[:, 0:w - 1],
                            op=mybir.AluOpType.add,
                        )

            act_slot = 0
            for ci, t in enumerate(OWN):
                w = _width(t)
                strip = strip_ps.tile([128, 33], f32, tag="s")
                e_buf = ebuf_pool.tile([128, 4224], bf16, tag="e")
                i16 = i16_pool.tile([128, PIECE], mybir.dt.int16, tag="i")
                lhsT = xT[:, t * 128:(t + 1) * 128]
                pieces = _pieces(t)
                for pi, (pw, col0) in enumerate(pieces):
                    ps = main_ps.tile([128, PIECE], f32, tag="ps")
                    off = 0
                    while off < pw:
                        bw = min(512, pw - off)
                        nc.tensor.matmul(
                            ps[:, off:off + bw],
                            lhsT,
                            xT[:, col0 + off: col0 + off + bw],
                            start=True, stop=True,
                        )
                        off += bw
                        # interleave a few deferred colsum MMs after every
                        # logits block: keeps PE array activity high (HAM)
                        # and spreads the latency bubbles
                        pump(4)
                    po = col0 - t * 128
                    kt0, kt1 = max(1, po // 128), (po + pw) // 128
                    if pi == DVE_PIECE and ci in DVE_CHUNKS:
                        # Schraudolph fast-exp on DVE: int16 bits ARE bf16 e
                        nc.vector.tensor_scalar(
                            i16[:, 0:pw], ps[:, 0:pw], SCH_A, SCH_B,
                            mybir.AluOpType.mult, mybir.AluOpType.add,
                        )
                        nc.vector.reduce_sum(
                            Zdve[:, ci:ci + 1], i16[:, 0:pw].bitcast(bf16),
                            axis=mybir.AxisListType.X)
                        for k in range(kt0, kt1):
                            pending.append(
                                ("M", strip,
                                 i16[:, k * 128 - po:(k + 1) * 128 - po]
                                 .bitcast(bf16), k - 1))
                    else:
                        # exact exp + row-sum on ACT; e lands in SBUF as bf16
                        nc.scalar.activation(
                            e_buf[:, po:po + pw], ps[:, 0:pw],
                            mybir.ActivationFunctionType.Exp, scale=INV_TAU,
                            accum_out=Zact[:, act_slot:act_slot + 1],
                        )
                        act_slot += 1
                        for k in range(kt0, kt1):
                            pending.append(
                                ("M", strip,
                                 e_buf[:, k * 128:(k + 1) * 128], k - 1))
                pending.append(("FIN", t, strip, w))
            while pending:
                pump(1)
            nc.sync.dma_start(colacc_dram[:], colacc_sb[:])
        nc.sync.dma_start(zact_dram[:], Zact[:])
        nc.sync.dma_start(zdve_dram[:], Zdve[:])

    if split_waits:
        _split_multiwait(nc)
    return nc


def _get_nc(split_waits=True):
    global _NC
    if _NC is None:
        _NC = _build_nc(split_waits)
    return _NC


def _prep(x, lab):
    """Host-side O(N) prep: normalize, transpose, rotate per core, Msum."""
    x = np.asarray(x, dtype=np.float32)
    xh = x / np.linalg.norm(x, axis=-1, keepdims=True)
    xb = xh.astype(ml_dtypes.bfloat16)
    xbf = xb.astype(np.float32)
    # class-sum matrix in f32, then bf16 [D, NCLS]
    msum = np.zeros((NCLS, D), dtype=np.float32)
    np.add.at(msum, lab, xbf)
    msum_b = msum.astype(ml_dtypes.bfloat16).astype(np.float32)
    in_maps = []
    for c in range(N_CORES):
        xl = np.roll(xb, -128 * c, axis=0)          # local chunk t = global t+c
        xt = np.ascontiguousarray(xl.T)             # [D, N] bf16
        xt_ext = np.concatenate([xt, xt[:, :XT_COLS - N]], axis=1)
        in_maps.append({"xT": np.ascontiguousarray(xt_ext)})
    return in_maps, xbf, msum_b


def _combine(results, lab, xbf, msum_b):
    lab = np.asarray(lab)
    rd = (xbf * xbf).sum(axis=1)                    # bf16 ||x_i||^2 in f32
    Z = np.zeros(N, dtype=np.float64)
    # positive-pair sums S on host: one [N,D]x[D,NCLS] matmul
    S = np.einsum("nd,nd->n", xbf, msum_b[lab]).astype(np.float64)
    for c in range(N_CORES):
        r = results[c]
        zact = np.asarray(r["zact"], dtype=np.float64)      # [128, slots]
        zdve = np.asarray(r["zdve"], dtype=np.float64)      # [128, 8]
        colacc = np.asarray(r["colacc"], dtype=np.float64)  # [128, 96]
        # untouched PSUM columns (no colsum matmul ever wrote them) hold junk
        for n in range(96):
            if not _touchers(n):
                colacc[:, n] = 0.0
        # fold the extended colacc columns back mod 64
        cs = colacc[:, :64].copy()
        cs[:, :32] += colacc[:, 64:96]
        # column-sum contributions: local row (n, p) -> global chunk (n+c)%64
        gchunk = (np.arange(NCH) + c) % NCH
        idx = (gchunk[None, :] * 128 + np.arange(128)[:, None])  # [128, 64]
        np.add.at(Z, idx.ravel(), cs.ravel())
        # row-sum contributions + F per own chunk
        slot = 0
        for ci, t in enumerate(OWN):
            dve = ci in DVE_CHUNKS               # that piece went to the DVE
            n_act = len(_pieces(t)) - (1 if dve else 0)
            rows = ((t + c) % NCH) * 128 + np.arange(128)
            Z[rows] += zact[:, slot:slot + n_act].sum(axis=1)
            if dve:
                Z[rows] += zdve[:, ci]
            slot += n_act
    Zx = Z - np.exp(INV_TAU * rd.astype(np.float64))    # exclude diagonal
    lnZ = np.log(Zx)
    cnt = np.bincount(lab, minlength=NCLS)
    P = cnt[lab] - 1
    valid = P > 0
    Sx = INV_TAU * (S - rd)                             # exclude diagonal
    per_row = Sx / np.maximum(P, 1) - lnZ
    loss = -per_row[valid].sum() / valid.sum()
    return np.array(loss, dtype=np.float32)


def kernel(feature_embeds, label_ids):
    from concourse.bass_utils import run_bass_kernel_spmd

    lab = np.asarray(label_ids)
    in_maps, xbf, msum_b = _prep(feature_embeds, lab)
    nc = _get_nc()
    res = run_bass_kernel_spmd(nc, in_maps, list(range(N_CORES)))
    return _combine(res.results, lab, xbf, msum_b)


def kernel_profiled(feature_embeds, label_ids):
    """Same as kernel(), but with NTFF tracing; returns (loss, exec_time_ns)."""
    print("ntff hook installed:", _install_ntff_hook())
    from concourse.bass_utils import run_bass_kernel_spmd

    lab = np.asarray(label_ids)
    in_maps, xbf, msum_b = _prep(feature_embeds, lab)
    nc = _get_nc()
    res = run_bass_kernel_spmd(
        nc, in_maps, list(range(N_CORES)), trace=True
    )
    return _combine(res.results, lab, xbf, msum_b), res.exec_time_ns


# revision 38
# speedup vs baseline: 1.1739x; 1.1739x over previous
"""Supervised contrastive loss (nn_Batch_CL) on 8 Trainium2 NeuronCores.

Math (per the reference):
  x = l2_normalize(feature_embeds)            # [N, D]
  logits = (x @ x.T) / tau                    # tau = 0.1
  Z_i    = sum_{j != i} exp(logits[i, j])
  S_i    = sum_{j != i, l_j == l_i} logits[i, j]
  per_row_i = S_i / P_i - log Z_i   (P_i = #positives, if > 0)
  loss = -sum(per_row) / n_valid

Strategy (v2): exploit the SYMMETRY of the logits matrix — only the upper
triangle of the 64x64 grid of [128,128] tiles is exp'd (half the N^2 ACT
work).  Each tile contributes its row-sums (ACT accum_out, free) to the Z of
its own rows AND its column-sums (per-tile e-as-stationary PE matmul with a
ones vector, out [128,1] per tile) to the Z of the mirrored rows.

Distribution: circulant chunk assignment.  Global chunk m (of 64) owns tiles
(m, m+d mod 64) for d=0..32 if m<32 else d=0..31 (each unordered chunk pair
covered exactly once).  Core c takes global chunks {c, c+8, ..., c+56}; its
input rows are rotated by 128*c so every core runs the IDENTICAL program on
local chunks {0,8,...,56}.  The wrap (mod 8192 columns) is removed by feeding
xT extended with a 3072-column copy of the first columns.

The exp work is split between ACT (exact spline exp, with free row-sums via
accum_out) and the DVE (Schraudolph bit-trick exp producing bf16 bit patterns
in an int16 tile, rel err ~±3% zero-mean — vanishes in the 8192-term Z sums).

Host does all O(N) work: l2-normalize + transpose + bf16 cast, the
positive-pair sums S = (x @ Msum^T)[i, lab_i] via the class-sum matrix, and
the final assembly (Z = row parts + unrotated col parts - exact diag, log,
positive counts, masked mean).  The device is pure N^2 compute: logits
matmuls, exp + row-sums, colsum matmuls.
"""

import numpy as np
import ml_dtypes

N = 8192
D = 128
N_CORES = 8
NCH = 64                         # global/local 128-row chunks
OWN = [0, 8, 16, 24, 32, 40, 48, 56]   # local chunk ids owned by every core
NOWN = len(OWN)
XT_COLS = N + 3072               # extended (wrap-free) xT width; max span end
PIECE = 1536                     # psum piece width (3 banks)
NCLS = 33
INV_TAU = 10.0

def _width(t):                   # tiles in chunk t's span, incl. diagonal tile
    return 33 if t < 32 else 32

# per-chunk static piece tables: list of (psum_width, col0).  The last chunk's
# tail is split in two so the end-of-kernel exp+colsum chain pipelines.
def _pieces(t):
    ws = _width(t) * 128
    sizes = [1536, 1536, 512, 512] if t == OWN[-1] else [1536, 1536, ws - 3072]
    out = []
    off = 0
    for w in sizes:
        w = min(w, ws - off)
        if w <= 0:
            break
        out.append((w, t * 128 + off))
        off += w
    return out

N_ACT_SLOTS = sum(len(_pieces(t)) for t in OWN)

# Schraudolph fast-exp on DVE: e^(10*s) ~= bitcast_bf16(int16(s*SCH_A + SCH_B)).
# bf16 bits = 128*(127 + log2 y); SCH_A = 128*10*log2(e); the -7.3344 centers
# the piecewise-linear log2 error to zero mean over a uniform mantissa.
SCH_A = 128.0 * INV_TAU * 1.4426950408889634
SCH_B = 128.0 * 127 - 7.3344
DVE_PIECE = 1         # piece index routed through the DVE fast-exp
DVE_CHUNKS = {1, 2, 3, 4, 5, 6}   # chunk indices whose piece 1 goes to DVE
                                  # (0 and 7 stay ACT: chunk 7's tail colsums
                                  # must not wait on the DVE pipeline)

def _touchers(n):
    """Own chunks whose span covers extended column-chunk n (no wrap)."""
    return [t for t in OWN if 1 <= n - t <= _width(t) - 1]

_NC = None

# ---------------------------------------------------------------------------
# Inlined workarounds (kernel.py must be self-contained).
# The local walrus build accepts at most ONE sync-wait per instruction; Tile
# attaches several.  Patch the drain barrier + hoist extra waits onto nops.
# ---------------------------------------------------------------------------

_nop_counter = [0]


def _split_drain_and_barrier(self, tick_clock, wait_clock):
    import bass_rust

    vec = tick_clock.global_clock  # VectorClock
    for proc in range(len(vec)):
        tickv = vec[proc]
        if tickv > 0:
            nop_inst = self.nc.sync.nop(nofuse=True)
            c = bass_rust.ScopedClock()
            c.require_at_least(None, proc, tickv)
            wait_clock.add_sem_waits(nop_inst.ins, c)
    self.nc.sync.drain()
    self.nc.all_engine_barrier()
    assert self.sems is not None
    popped = self.nc._tile_sem_poison_stack.pop()
    assert popped is self._sem_poison
    self.nc.clear_and_free_semaphores(list(self.sems.allocated().values()))
    self.nc.all_engine_barrier()


def _install_tile_patch():
    from concourse import tile as _tile

    _tile.TileContext._drain_and_barrier = _split_drain_and_barrier


def _split_multiwait(nc):
    """Hoist all-but-one sync wait from every instruction onto nops."""
    import concourse.mybir as mybir

    n_hoisted = 0
    for bb in nc.main_func.blocks:
        insns = bb.instructions
        out = []
        changed = False
        for ins in insns:
            si = ins.sync_info
            if si is not None and len(si.on_wait) > 1:
                waits = list(si.on_wait)
                for w in waits[:-1]:
                    _nop_counter[0] += 1
                    nop = mybir.InstEventSemaphore(
                        name=f"hoistnop-{_nop_counter[0]}",
                        engine=ins.engine,
                        sync_info=mybir.SyncInfo(on_wait=[w], on_update=[]),
                    )
                    out.append(nop)
                    n_hoisted += 1
                ins.sync_info = mybir.SyncInfo(
                    on_wait=[waits[-1]], on_update=list(si.on_update)
                )
                changed = True
            out.append(ins)
        if changed:
            bb.instructions = out
    return n_hoisted


def _install_ntff_hook():
    """Synthesize the antenv.axon_hooks module missing from this image so
    run_bass_kernel_spmd(trace=True) can NTFF-profile under axon."""
    import sys
    import types

    if "antenv.axon_hooks" in sys.modules:
        return True
    try:
        import antenv
        from trn_agent_boot.trn_boot import _ntff_profile_via_ctypes
    except ImportError:
        return False
    hook_box = [None]
    mod = types.ModuleType("antenv.axon_hooks")
    mod.set_axon_ntff_profile_hook = lambda h: hook_box.__setitem__(0, h)
    mod.get_axon_ntff_profile_hook = lambda: hook_box[0]
    sys.modules["antenv.axon_hooks"] = mod
    antenv.axon_hooks = mod
    hook = _ntff_profile_via_ctypes("/opt/axon/libaxon_pjrt.so")
    mod.set_axon_ntff_profile_hook(hook)
    return hook is not None


def _build_nc(split_waits=True):
    import concourse.bass as bass
    import concourse.mybir as mybir
    from concourse import tile
    from contextlib import ExitStack

    _install_tile_patch()

    f32 = mybir.dt.float32
    bf16 = mybir.dt.bfloat16

    nc = bass.Bass()
    xT_dram = nc.dram_tensor("xT", [128, XT_COLS], bf16, kind="ExternalInput")
    zact_dram = nc.dram_tensor("zact", [128, N_ACT_SLOTS], f32, kind="ExternalOutput")
    zdve_dram = nc.dram_tensor("zdve", [128, NOWN], f32, kind="ExternalOutput")
    colacc_dram = nc.dram_tensor("colacc", [128, 96], f32, kind="ExternalOutput")

    with tile.TileContext(nc) as tc, ExitStack() as ctx:
        persist = ctx.enter_context(tc.tile_pool(name="persist", bufs=1))

        xT = persist.tile([128, XT_COLS], bf16)
        ones_bf = persist.tile([128, 1], bf16)
        colacc_sb = persist.tile([128, 96], f32)
        Zact = persist.tile([128, N_ACT_SLOTS], f32)
        Zdve = persist.tile([128, NOWN], f32)

        # ---------------- prologue ----------------
        # finer input DMA pieces alternating over the two hardware-DGE queues
        # (SP + ACT); gpsimd's software DGE would contend the DVE SBUF port
        bounds = [0, 512, 1536, 3072, 4608, 6144, 8192, 9728, XT_COLS]
        dma_engines = [nc.sync, nc.scalar]
        for k in range(len(bounds) - 1):
            dma_engines[k % 2].dma_start(
                xT[:, bounds[k]:bounds[k + 1]],
                xT_dram[:, bounds[k]:bounds[k + 1]])
        nc.vector.memset(ones_bf[:], 1.0)

        # ---------------- main loop ----------------
        with (
            tc.tile_pool(name="main_ps", bufs=2, space="PSUM") as main_ps,
            tc.tile_pool(name="strip_ps", bufs=2, space="PSUM") as strip_ps,
            tc.tile_pool(name="ebuf", bufs=2) as ebuf_pool,
            tc.tile_pool(name="i16", bufs=2) as i16_pool,
        ):
            # warm the PE (HAM clock gate) with bare weight loads (junk data,
            # no PSUM use) while the input DMA streams in
            garb = ebuf_pool.tile([128, 128], bf16, tag="warm")
            nc.vector.memset(garb[:], 1.0)
            for _ in range(56):
                nc.tensor.ldweights(weights=garb[:])
            nc.vector.memset(colacc_sb[:], 0.0)

            pending = []  # deferred PE work, finest grain:
                          # ("M", strip, src_ap, col)  one colsum matmul
                          # ("FIN", t, strip, w)       DVE add of a full strip

            def pump(nitems):
                for _ in range(nitems):
                    if not pending:
                        return
                    item = pending.pop(0)
                    if item[0] == "M":
                        _, strip, src_ap, sc = item
                        nc.tensor.matmul(
                            strip[:, sc:sc + 1], src_ap, ones_bf[:],
                            start=True, stop=True,
                        )
                    else:
                        _, t, strip, w = item
                        # colacc[:, t+1 : t+w] += strip
                        nc.vector.tensor_tensor(
                            out=colacc_sb[:, t + 1:t + w],
                            in0=colacc_sb[:, t + 1:t + w],
                            in1=strip[:, 0:w - 1],
                            op=mybir.AluOpType.add,
                        )

            act_slot = 0
            for ci, t in enumerate(OWN):
                w = _width(t)
                strip = strip_ps.tile([128, 33], f32, tag="s")
                e_buf = ebuf_pool.tile([128, 4224], bf16, tag="e")
                i16 = i16_pool.tile([128, PIECE], mybir.dt.int16, tag="i")
                lhsT = xT[:, t * 128:(t + 1) * 128]
                pieces = _pieces(t)
                for pi, (pw, col0) in enumerate(pieces):
                    ps = main_ps.tile([128, PIECE], f32, tag="ps")
                    off = 0
                    while off < pw:
                        bw = min(512, pw - off)
                        nc.tensor.matmul(
                            ps[:, off:off + bw],
                            lhsT,
                            xT[:, col0 + off: col0 + off + bw],
                            start=True, stop=True,
                        )
                        off += bw
                        # interleave a few deferred colsum MMs after every
                        # logits block: keeps PE array activity high (HAM)
                        # and spreads the latency bubbles
                        pump(4)
                    po = col0 - t * 128
                    kt0, kt1 = max(1, po // 128), (po + pw) // 128
                    if pi == DVE_PIECE and ci in DVE_CHUNKS:
                        # Schraudolph fast-exp on DVE: int16 bits ARE bf16 e
                        nc.vector.tensor_scalar(
                            i16[:, 0:pw], ps[:, 0:pw], SCH_A, SCH_B,
                            mybir.AluOpType.mult, mybir.AluOpType.add,
                        )
                        nc.vector.reduce_sum(
                            Zdve[:, ci:ci + 1], i16[:, 0:pw].bitcast(bf16),
                            axis=mybir.AxisListType.X)
                        for k in range(kt0, kt1):
                            pending.append(
                                ("M", strip,
                                 i16[:, k * 128 - po:(k + 1) * 128 - po]
                                 .bitcast(bf16), k - 1))
                    else:
                        # exact exp + row-sum on ACT; e lands in SBUF as bf16
                        nc.scalar.activation(
                            e_buf[:, po:po + pw], ps[:, 0:pw],
                            mybir.ActivationFunctionType.Exp, scale=INV_TAU,
                            accum_out=Zact[:, act_slot:act_slot + 1],
                        )
                        act_slot += 1
                        for k in range(kt0, kt1):
                            pending.append(
                                ("M", strip,
                                 e_buf[:, k * 128:(k + 1) * 128], k - 1))
                pending.append(("FIN", t, strip, w))
            while pending:
                pump(1)
            nc.sync.dma_start(colacc_dram[:], colacc_sb[:])
        nc.sync.dma_start(zact_dram[:], Zact[:])
        nc.sync.dma_start(zdve_dram[:], Zdve[:])

    if split_waits:
        _split_multiwait(nc)
    return nc


def _get_nc(split_waits=True):
    global _NC
    if _NC is None:
        _NC = _build_nc(split_waits)
    return _NC


def _prep(x, lab):
    """Host-side O(N) prep: normalize, transpose, rotate per core, Msum."""
    x = np.asarray(x, dtype=np.float32)
    xh = x / np.linalg.norm(x, axis=-1, keepdims=True)
    xb = xh.astype(ml_dtypes.bfloat16)
    xbf = xb.astype(np.float32)
    # class-sum matrix in f32, then bf16 [D, NCLS]
    msum = np.zeros((NCLS, D), dtype=np.float32)
    np.add.at(msum, lab, xbf)
    msum_b = msum.astype(ml_dtypes.bfloat16).astype(np.float32)
    in_maps = []
    for c in range(N_CORES):
        xl = np.roll(xb, -128 * c, axis=0)          # local chunk t = global t+c
        xt = np.ascontiguousarray(xl.T)             # [D, N] bf16
        xt_ext = np.concatenate([xt, xt[:, :XT_COLS - N]], axis=1)
        in_maps.append({"xT": np.ascontiguousarray(xt_ext)})
    return in_maps, xbf, msum_b


def _combine(results, lab, xbf, msum_b):
    lab = np.asarray(lab)
    rd = (xbf * xbf).sum(axis=1)                    # bf16 ||x_i||^2 in f32
    Z = np.zeros(N, dtype=np.float64)
    # positive-pair sums S on host: one [N,D]x[D,NCLS] matmul
    S = np.einsum("nd,nd->n", xbf, msum_b[lab]).astype(np.float64)
    for c in range(N_CORES):
        r = results[c]
        zact = np.asarray(r["zact"], dtype=np.float64)      # [128, slots]
        zdve = np.asarray(r["zdve"], dtype=np.float64)      # [128, 8]
        colacc = np.asarray(r["colacc"], dtype=np.float64)  # [128, 96]
        # untouched PSUM columns (no colsum matmul ever wrote them) hold junk
        for n in range(96):
            if not _touchers(n):
                colacc[:, n] = 0.0
        # fold the extended colacc columns back mod 64
        cs = colacc[:, :64].copy()
        cs[:, :32] += colacc[:, 64:96]
        # column-sum contributions: local row (n, p) -> global chunk (n+c)%64
        gchunk = (np.arange(NCH) + c) % NCH
        idx = (gchunk[None, :] * 128 + np.arange(128)[:, None])  # [128, 64]
        np.add.at(Z, idx.ravel(), cs.ravel())
        # row-sum contributions + F per own chunk
        slot = 0
        for ci, t in enumerate(OWN):
            dve = ci in DVE_CHUNKS               # that piece went to the DVE
            n_act = len(_pieces(t)) - (1 if dve else 0)
            rows = ((t + c) % NCH) * 128 + np.arange(128)
            Z[rows] += zact[:, slot:slot + n_act].sum(axis=1)
            if dve:
                Z[rows] += zdve[:, ci]
            slot += n_act
    Zx = Z - np.exp(INV_TAU * rd.astype(np.float64))    # exclude diagonal
    lnZ = np.log(Zx)
    cnt = np.bincount(lab, minlength=NCLS)
    P = cnt[lab] - 1
    valid = P > 0
    Sx = INV_TAU * (S - rd)                             # exclude diagonal
    per_row = Sx / np.maximum(P, 1) - lnZ
    loss = -per_row[valid].sum() / valid.sum()
    return np.array(loss, dtype=np.float32)


def kernel(feature_embeds, label_ids):
    from concourse.bass_utils import run_bass_kernel_spmd

    lab = np.asarray(label_ids)
    in_maps, xbf, msum_b = _prep(feature_embeds, lab)
    nc = _get_nc()
    res = run_bass_kernel_spmd(nc, in_maps, list(range(N_CORES)))
    return _combine(res.results, lab, xbf, msum_b)


def kernel_profiled(feature_embeds, label_ids):
    """Same as kernel(), but with NTFF tracing; returns (loss, exec_time_ns)."""
    print("ntff hook installed:", _install_ntff_hook())
    from concourse.bass_utils import run_bass_kernel_spmd

    lab = np.asarray(label_ids)
    in_maps, xbf, msum_b = _prep(feature_embeds, lab)
    nc = _get_nc()
    res = run_bass_kernel_spmd(
        nc, in_maps, list(range(N_CORES)), trace=True
    )
    return _combine(res.results, lab, xbf, msum_b), res.exec_time_ns
